# revision 35
# baseline (speedup 1.0000x reference)
"""Trainium2 Bass kernel for point-cloud GRU (kNN set-conv gates, InstanceNorm).

Strategy (8 cores, B=2):
  - 4 cores per batch, each owning a 1024-point shard of S=4096.
  - kNN (k=4): PE computes score[i,j] = |x_j|^2 - 2 x_i.x_j for own rows,
    DVE max8+max_index on negated scores -> 4 smallest (self included).
  - Set-conv is linearized: y[s,k,o] = w[idx[s,k], o] + c[o, s] where
    w[n,o] = W_feat.f[n] + W_xyz.xyz[n] (per-point projection table) and
    c[o,s] = b[o] - W_xyz.xyz[s].  Each core builds only the table rows
    for its OWN 1024 points (from its own h/x shard), then the group of 4
    AllGathers the full [S, 3*O] fp16 table; rows gathered by kNN index
    (SWDGE indirect DMA).
  - InstanceNorm stats over (S,k) per (b,o) from algebraic identities:
      sum y   = A + k*Cs,   A  = sum_s t[s],  t = sum_k w[idx[s,k]]
      sum y^2 = B2 + 2*X + k*C2,  B2 = sum_s sum_k w^2,  X = sum_s c.t
    A/B2/X via PE ones-matmuls; Cs/C2 via ScalarE accum; partials
    AllReduced across the 4-core batch group (tiny).
  - max_k commutes with the (monotonic) normalization: out uses m = max_k w.
  - Phase 2 (q gate): each core projects its own r*h shard through Wq_h
    into its q-table slab, AllGather -> full [S, O] table, gather+fold.

Host side: the jitted shard_map executable is compiled ONCE (AOT, fast
dispatch) and cached; repeat calls only do cheap numpy prep + transfer.
All bulk inputs ship as fp16 shards (each core gets only its own points),
so per-call wire traffic is ~10 MB instead of ~58 MB.
"""

import numpy as np

B, S, H, D = 2, 4096, 128, 256
O = 128
K = 4
NCORES = 8
GROUP = 4              # cores per batch
PTS = S // GROUP       # points per core
ST = PTS // 128        # 8 own s-tiles
EPS = 1e-5
NK = float(S * K)

_CACHE = {}


def _build_program():
    from concourse import bass, bacc, mybir, tile
    from concourse.masks import make_identity

    dt = mybir.dt
    f32, f16, u32 = dt.float32, dt.float16, dt.uint32
    f8x, f8o = dt.float8e4, dt.float8e3
    AF = mybir.ActivationFunctionType
    ALU = mybir.AluOpType

    nc = bacc.Bacc("TRN2", target_bir_lowering=False, debug=False,
                   enable_asserts=False, num_devices=NCORES)

    # ---------------- I/O (per-core shards; fp16 for bulk) ----------------
    # Replicated weights ship SHARDED (1/8 each) and are AllGathered on
    # device, so the wire carries exactly one copy of each weight.
    feat = nc.dram_tensor("feat", [3 * 128, PTS], f16, kind="ExternalInput").ap()
    pct_in = nc.dram_tensor("pct", [4, PTS], f32, kind="ExternalInput").ap()
    pcm_in = nc.dram_tensor("pcm", [4, PTS], f32, kind="ExternalInput").ap()
    wtsh = nc.dram_tensor("wtsh", [48, 3 * O], f16, kind="ExternalInput").ap()
    wtg = nc.dram_tensor("wtg", [3, 3 * O], f16, kind="ExternalInput").ap()
    wqsh = nc.dram_tensor("wqsh", [16, O], f16, kind="ExternalInput").ap()
    bcol = nc.dram_tensor("bcol", [128, 3], f32, kind="ExternalInput").ap()
    brow = nc.dram_tensor("brow", [1, 3 * O], f32, kind="ExternalInput").ap()
    out_io = nc.dram_tensor("out", [NCORES, O, PTS], f8o,
                            kind="ExternalOutput").ap()

    # ---------------- internal DRAM ----------------
    # (collectives may not read/write IO tensors -> internal staging copies)
    wtsh_st = nc.dram_tensor("wtsh_st", [48, 3 * O], f16, kind="Internal").ap()
    wqsh_st = nc.dram_tensor("wqsh_st", [16, O], f16, kind="Internal").ap()
    pcm_st = nc.dram_tensor("pcm_st", [4, PTS], f32, kind="Internal").ap()
    out_st = nc.dram_tensor("out_st", [NCORES, O, PTS], f8o,
                            kind="Internal").ap()
    wts_full = nc.dram_tensor("wts_full", [3 * 128, 3 * O], f16,
                              kind="Internal").ap()
    wqh_full = nc.dram_tensor("wqh_full", [128, O], f16, kind="Internal").ap()
    pca_blk = nc.dram_tensor("pca_blk", [GROUP, 4, PTS], f32,
                             kind="Internal").ap()
    tbm1 = nc.dram_tensor("tbm1", [PTS, 3 * O], f16, kind="Internal").ap()
    tb1 = nc.dram_tensor("tb1", [S, 3 * O], f16, kind="Internal").ap()
    tbm2 = nc.dram_tensor("tbm2", [PTS, O], f16, kind="Internal").ap()
    tb2 = nc.dram_tensor("tb2", [S, O], f16, kind="Internal").ap()
    outm = nc.dram_tensor("outm", [O, PTS], f8o, kind="Internal").ap()
    cc1_in = nc.dram_tensor("cc1_in", [128, 10], f32, kind="Internal").ap()
    cc1_out = nc.dram_tensor("cc1_out", [128, 10], f32, kind="Internal").ap()
    cc2_in = nc.dram_tensor("cc2_in", [128, 5], f32, kind="Internal").ap()
    cc2_out = nc.dram_tensor("cc2_out", [128, 5], f32, kind="Internal").ap()

    RG = [[0, 1, 2, 3], [4, 5, 6, 7]]
    RG8 = [[0, 1, 2, 3, 4, 5, 6, 7]]

    from contextlib import ExitStack
    ctx = ExitStack()
    with tile.TileContext(nc) as tc, ctx:
        persist = ctx.enter_context(tc.tile_pool(name="persist", bufs=1))
        gst_pool = ctx.enter_context(tc.tile_pool(name="gst", bufs=1))
        sc_pool = ctx.enter_context(tc.tile_pool(name="scores", bufs=2))
        wk_pool = ctx.enter_context(tc.tile_pool(name="work", bufs=2))
        ps_pool = ctx.enter_context(tc.tile_pool(name="ps", bufs=6, space="PSUM"))
        px_pool = ctx.enter_context(tc.tile_pool(name="psX", bufs=1, space="PSUM"))

        def psum(shape, tag="ps", dtp=None):
            return ps_pool.tile(shape, dtp or f32, tag=tag, name=tag)

        # ---- persistent SBUF ----
        h16 = persist.tile([128, PTS], f16)
        x016 = persist.tile([128, PTS], f16)
        x116 = persist.tile([128, PTS], f16)
        hmy32 = persist.tile([128, PTS], f32)
        pca_sb = persist.tile([4, S], f32)
        pct_sb = persist.tile([4, PTS], f32)
        pcm_sb = persist.tile([4, PTS], f32)
        pcm16 = persist.tile([4, PTS], f16)
        wt0_sb = persist.tile([128, 3 * O], f16)
        wt1_sb = persist.tile([128, 3 * O], f16)
        wt2_sb = persist.tile([128, 3 * O], f16)
        wtg_sb = persist.tile([3, 3 * O], f16)
        wqh_sb = persist.tile([128, O], f16)
        bcol_sb = persist.tile([128, 3], f32)
        brow_sb = persist.tile([1, 3 * O], f32)
        idx_sb = persist.tile([128, 8 * ST], u32)
        ones16 = persist.tile([128, 1], f16)
        onesK = persist.tile([1, 128], f32)
        ident = persist.tile([128, 128], f16)
        b_bc = persist.tile([128, 3 * O], f16)
        c_cm = persist.tile([128, 3 * PTS], f16)        # c channel-major, per gate
        csum = persist.tile([128, 12], f32)             # Cs/C2 halves per gate
        m_cm = persist.tile([128, 3 * PTS], f16)        # gathered-max, channel-major
        c_pm = [persist.tile([128, 3 * O], f16, tag=f"c_pm{i}", name=f"c_pm{i}")
                for i in range(ST)]
        stats_sb = persist.tile([128, 10], f32)
        scl = persist.tile([128, 8], f32)               # istd/nbias per gate
        z_sb = persist.tile([O, PTS], f32)
        r_sb = persist.tile([O, PTS], f32)

        stats_ps = px_pool.tile([128, 96], f32)         # PE stat columns

        # weight shards -> full copies via 8-wide AllGather (device links
        # are far faster than the host tunnel); pca built from the group
        # AllGather of each core's own [xyz; |x|^2] block.
        nc.sync.dma_start(out=wtsh_st, in_=wtsh)
        nc.sync.dma_start(out=wqsh_st, in_=wqsh)
        nc.sync.dma_start(out=pcm_st, in_=pcm_in)
        nc.gpsimd.collective_compute("AllGather", mybir.AluOpType.bypass,
                                     replica_groups=RG8,
                                     ins=[wtsh_st], outs=[wts_full])
        nc.gpsimd.collective_compute("AllGather", mybir.AluOpType.bypass,
                                     replica_groups=RG8,
                                     ins=[wqsh_st], outs=[wqh_full])
        nc.gpsimd.collective_compute("AllGather", mybir.AluOpType.bypass,
                                     replica_groups=RG,
                                     ins=[pcm_st], outs=[pca_blk])
        nc.sync.dma_start(out=h16, in_=feat[0:128, :])
        nc.sync.dma_start(out=x016, in_=feat[128:256, :])
        nc.sync.dma_start(out=x116, in_=feat[256:384, :])
        nc.sync.dma_start(out=pct_sb, in_=pct_in)
        nc.sync.dma_start(out=pcm_sb, in_=pcm_in)
        for g in range(GROUP):
            nc.sync.dma_start(out=pca_sb[:, g * PTS:(g + 1) * PTS],
                              in_=pca_blk[g])
        nc.sync.dma_start(out=wt0_sb, in_=wts_full[0:128, :])
        nc.sync.dma_start(out=wt1_sb, in_=wts_full[128:256, :])
        nc.sync.dma_start(out=wt2_sb, in_=wts_full[256:384, :])
        nc.sync.dma_start(out=wtg_sb, in_=wtg)
        nc.sync.dma_start(out=wqh_sb, in_=wqh_full)
        nc.sync.dma_start(out=bcol_sb, in_=bcol)
        nc.sync.dma_start(out=brow_sb, in_=brow)

        nc.vector.memset(ones16, 1.0)
        nc.vector.memset(onesK, 1.0)
        make_identity(nc, ident[:])
        # fp16 copies for the table xyz contribution
        nc.scalar.activation(out=pcm16, in_=pcm_sb, func=AF.Copy)
        nc.scalar.activation(out=hmy32, in_=h16, func=AF.Copy)

        # b broadcast down partitions (point-major bias): ones^T @ brow
        psb = psum([128, 3 * O])
        nc.tensor.matmul(out=psb, lhsT=onesK, rhs=brow_sb, start=True, stop=True)
        nc.scalar.activation(out=b_bc, in_=psb, func=AF.Copy)

        # ---- own-slab table (z | r | q-static): 8 M-tiles, then AllGather ----
        for mt in range(ST):
            sl = slice(mt * 128, (mt + 1) * 128)
            pst = psum([128, 3 * O])
            nc.tensor.matmul(out=pst, lhsT=h16[:, sl], rhs=wt0_sb,
                             start=True, stop=False)
            nc.tensor.matmul(out=pst, lhsT=x016[:, sl], rhs=wt1_sb,
                             start=False, stop=False)
            nc.tensor.matmul(out=pst, lhsT=x116[:, sl], rhs=wt2_sb,
                             start=False, stop=False)
            nc.tensor.matmul(out=pst, lhsT=pcm16[0:3, sl], rhs=wtg_sb,
                             start=False, stop=True)
            tb_sb = wk_pool.tile([128, 3 * O], f16, tag="tb_sb", name="tb_sb")
            nc.scalar.activation(out=tb_sb, in_=pst, func=AF.Copy)
            nc.sync.dma_start(out=tbm1[sl, :], in_=tb_sb)
        nc.gpsimd.collective_compute("AllGather", mybir.AluOpType.bypass,
                                     replica_groups=RG,
                                     ins=[tbm1], outs=[tb1])

        # ---- scores + top-4 (overlaps the table AllGather) ----
        for st in range(ST):
            srow = sc_pool.tile([128, S], f32, tag="srow", name="srow")
            for ch in range(8):
                ps = psum([128, 512])
                nc.tensor.matmul(out=ps,
                                 lhsT=pct_sb[:, st * 128:(st + 1) * 128],
                                 rhs=pca_sb[:, ch * 512:(ch + 1) * 512],
                                 start=True, stop=True)
                # negate so max8 finds the smallest distances
                nc.scalar.activation(out=srow[:, ch * 512:(ch + 1) * 512],
                                     in_=ps, func=AF.Copy, scale=-1.0)
            mx = wk_pool.tile([128, 8], f32, tag="mx8", name="mx8")
            nc.vector.max(out=mx, in_=srow)
            nc.vector.max_index(out=idx_sb[:, st * 8:st * 8 + 8],
                                in_max=mx, in_values=srow)

        # ---------------- c tiles ----------------
        # channel-major: c[o, s] = b[o] - v[o, s];  Cs/C2 via ScalarE accum.
        for g in range(3):
            for hh in range(2):
                psv = psum([128, 512])
                nc.tensor.matmul(out=psv,
                                 lhsT=wtg_sb[:, g * O:(g + 1) * O],
                                 rhs=pcm16[0:3, hh * 512:(hh + 1) * 512],
                                 start=True, stop=True)
                cs = slice(g * PTS + hh * 512, g * PTS + (hh + 1) * 512)
                nc.scalar.activation(out=c_cm[:, cs], in_=psv, func=AF.Identity,
                                     bias=bcol_sb[:, g:g + 1], scale=-1.0,
                                     accum_out=csum[:, 4 * g + hh:4 * g + hh + 1])
                scr = wk_pool.tile([128, 512], f16, tag="c2scr")
                nc.scalar.activation(out=scr, in_=psv, func=AF.Square,
                                     bias=bcol_sb[:, g:g + 1], scale=-1.0,
                                     accum_out=csum[:, 4 * g + 2 + hh:4 * g + 3 + hh])

        # point-major c tiles (for the X statistic)
        for st in range(ST):
            psv2 = psum([128, 3 * O])
            nc.tensor.matmul(out=psv2,
                             lhsT=pcm16[0:3, st * 128:(st + 1) * 128],
                             rhs=wtg_sb, start=True, stop=True)
            nc.scalar.activation(out=c_pm[st], in_=psv2, func=AF.Copy, scale=-1.0)
            nc.vector.tensor_add(c_pm[st], c_pm[st], b_bc)

        # ---------------- phase-1 gathers + folds (z, r) ----------------
        gtiles = [[gst_pool.tile([128, 3 * O], f16, tag=f"g{st}_{j}",
                              name=f"g{st}_{j}") for j in range(K)]
                  for st in range(ST)]
        for st in range(ST):
            g0, g1, g2, g3 = gtiles[st]
            for j in range(K):
                nc.gpsimd.indirect_dma_start(
                    out=gtiles[st][j][:], out_offset=None, in_=tb1[:, :],
                    in_offset=bass.IndirectOffsetOnAxis(
                        ap=idx_sb[:, st * 8 + j:st * 8 + j + 1], axis=0))
            zr = slice(0, 2 * O)
            t = wk_pool.tile([128, 2 * O], f16, tag="t_zr")
            nc.vector.tensor_add(t, g0[:, zr], g1[:, zr])
            nc.vector.tensor_add(t, t, g2[:, zr])
            nc.vector.tensor_add(t, t, g3[:, zr])
            m = wk_pool.tile([128, 2 * O], f16, tag="m_zr")
            nc.vector.tensor_max(m, g0[:, zr], g1[:, zr])
            nc.vector.tensor_max(m, m, g2[:, zr])
            nc.vector.tensor_max(m, m, g3[:, zr])
            t2 = wk_pool.tile([128, 2 * O], f16, tag="t2_zr")
            sq = wk_pool.tile([128, 2 * O], f16, tag="sq_zr")
            nc.scalar.activation(out=t2, in_=g0[:, zr], func=AF.Square)
            nc.scalar.activation(out=sq, in_=g1[:, zr], func=AF.Square)
            nc.vector.tensor_add(t2, t2, sq)
            nc.scalar.activation(out=sq, in_=g2[:, zr], func=AF.Square)
            nc.vector.tensor_add(t2, t2, sq)
            nc.scalar.activation(out=sq, in_=g3[:, zr], func=AF.Square)
            nc.vector.tensor_add(t2, t2, sq)
            ct = wk_pool.tile([128, 2 * O], f16, tag="ct_zr")
            nc.vector.tensor_mul(ct, c_pm[st][:, zr], t)
            for qi, srct in ((0, t), (2, t2), (4, ct)):
                for gx in range(2):
                    col = (qi + gx) * 8 + st
                    nc.tensor.matmul(out=stats_ps[:, col:col + 1],
                                     lhsT=srct[:, gx * O:(gx + 1) * O],
                                     rhs=ones16, start=True, stop=True)
            # transpose m -> channel-major
            for gx in range(2):
                ptr = psum([128, 128], dtp=f16)
                nc.tensor.transpose(out=ptr, in_=m[:, gx * O:(gx + 1) * O],
                                    identity=ident)
                nc.scalar.activation(
                    out=m_cm[:, gx * PTS + st * 128:gx * PTS + (st + 1) * 128],
                    in_=ptr, func=AF.Copy)

        # ---------------- stats AllReduce #1 (z, r) ----------------
        ccp = persist.tile([128, 10], f32)
        # cols: A B2 X Cs C2 per gate
        for gx in range(2):
            nc.vector.tensor_reduce(out=ccp[:, 5 * gx + 0:5 * gx + 1],
                                    in_=stats_ps[:, (0 + gx) * 8:(0 + gx) * 8 + 8],
                                    axis=mybir.AxisListType.X, op=ALU.add)
            nc.vector.tensor_reduce(out=ccp[:, 5 * gx + 1:5 * gx + 2],
                                    in_=stats_ps[:, (2 + gx) * 8:(2 + gx) * 8 + 8],
                                    axis=mybir.AxisListType.X, op=ALU.add)
            nc.vector.tensor_reduce(out=ccp[:, 5 * gx + 2:5 * gx + 3],
                                    in_=stats_ps[:, (4 + gx) * 8:(4 + gx) * 8 + 8],
                                    axis=mybir.AxisListType.X, op=ALU.add)
            nc.vector.tensor_add(ccp[:, 5 * gx + 3:5 * gx + 4],
                                 csum[:, 4 * gx:4 * gx + 1],
                                 csum[:, 4 * gx + 1:4 * gx + 2])
            nc.vector.tensor_add(ccp[:, 5 * gx + 4:5 * gx + 5],
                                 csum[:, 4 * gx + 2:4 * gx + 3],
                                 csum[:, 4 * gx + 3:4 * gx + 4])
        nc.sync.dma_start(out=cc1_in, in_=ccp)
        nc.gpsimd.collective_compute("AllReduce", mybir.AluOpType.add,
                                     replica_groups=RG,
                                     ins=[cc1_in], outs=[cc1_out])
        nc.sync.dma_start(out=stats_sb, in_=cc1_out)

        # ---------------- finalize gate scale/bias ----------------
        def finalize(gx, A, B2, X, Cs, C2, o_istd, o_nbias):
            w1 = wk_pool.tile([128, 1], f32, tag="fw1")
            w2 = wk_pool.tile([128, 1], f32, tag="fw2")
            w3 = wk_pool.tile([128, 1], f32, tag="fw3")
            # mu = (A + 4*Cs)/NK
            nc.vector.tensor_scalar(w1, Cs, 4.0, None, op0=ALU.mult)
            nc.vector.tensor_add(w1, w1, A)
            nc.vector.tensor_scalar(w1, w1, 1.0 / NK, None, op0=ALU.mult)
            # Ey2 = (B2 + 2X + 4*C2)/NK
            nc.vector.tensor_scalar(w2, X, 2.0, None, op0=ALU.mult)
            nc.vector.tensor_add(w2, w2, B2)
            nc.vector.tensor_scalar(w3, C2, 4.0, None, op0=ALU.mult)
            nc.vector.tensor_add(w2, w2, w3)
            nc.vector.tensor_scalar(w2, w2, 1.0 / NK, None, op0=ALU.mult)
            # var = Ey2 - mu^2 ; istd = 1/sqrt(var+eps); nbias = -mu*istd
            nc.vector.tensor_mul(w3, w1, w1)
            nc.vector.tensor_sub(w2, w2, w3)
            nc.vector.tensor_scalar_add(w2, w2, EPS)
            nc.scalar.activation(out=w2, in_=w2, func=AF.Sqrt)
            nc.vector.reciprocal(o_istd, w2)
            nc.vector.tensor_mul(o_nbias, w1, o_istd)
            nc.vector.tensor_scalar(o_nbias, o_nbias, -1.0, None, op0=ALU.mult)

        for gx in range(2):
            c0 = 5 * gx
            finalize(gx,
                     stats_sb[:, c0:c0 + 1], stats_sb[:, c0 + 1:c0 + 2],
                     stats_sb[:, c0 + 2:c0 + 3], stats_sb[:, c0 + 3:c0 + 4],
                     stats_sb[:, c0 + 4:c0 + 5],
                     scl[:, 2 * gx:2 * gx + 1], scl[:, 2 * gx + 1:2 * gx + 2])

        # ---------------- z, r gates ----------------
        for gx, dst in ((0, z_sb), (1, r_sb)):
            pre = wk_pool.tile([128, PTS], f16, tag="pre")
            nc.vector.tensor_add(pre, m_cm[:, gx * PTS:(gx + 1) * PTS],
                                 c_cm[:, gx * PTS:(gx + 1) * PTS])
            nc.scalar.activation(out=dst, in_=pre, func=AF.Sigmoid,
                                 scale=scl[:, 2 * gx:2 * gx + 1],
                                 bias=scl[:, 2 * gx + 1:2 * gx + 2])

        # ---------------- q table (dynamic part): own slab + AllGather ----
        rh = wk_pool.tile([H, PTS], f16, tag="rh")
        nc.vector.tensor_mul(rh, r_sb, hmy32)
        for mt in range(ST):
            sl = slice(mt * 128, (mt + 1) * 128)
            ps2 = psum([128, O])
            nc.tensor.matmul(out=ps2, lhsT=rh[:, sl], rhs=wqh_sb,
                             start=True, stop=True)
            tq_sb = wk_pool.tile([128, O], f16, tag="tq_sb")
            nc.scalar.activation(out=tq_sb, in_=ps2, func=AF.Copy)
            nc.sync.dma_start(out=tbm2[sl, :], in_=tq_sb)
        nc.gpsimd.collective_compute("AllGather", mybir.AluOpType.bypass,
                                     replica_groups=RG,
                                     ins=[tbm2], outs=[tb2])

        # ---------------- phase-2 gathers + folds (q) ----------------
        qs = slice(2 * O, 3 * O)
        for st in range(ST):
            gq = [wk_pool.tile([128, O], f16, tag=f"gq{j}", name=f"gq{j}")
                  for j in range(K)]
            for j in range(K):
                nc.gpsimd.indirect_dma_start(
                    out=gq[j][:], out_offset=None, in_=tb2[:, :],
                    in_offset=bass.IndirectOffsetOnAxis(
                        ap=idx_sb[:, st * 8 + j:st * 8 + j + 1], axis=0))
                nc.vector.tensor_add(gq[j], gq[j], gtiles[st][j][:, qs])
            t = wk_pool.tile([128, O], f16, tag="t_q")
            nc.vector.tensor_add(t, gq[0], gq[1])
            nc.vector.tensor_add(t, t, gq[2])
            nc.vector.tensor_add(t, t, gq[3])
            m = wk_pool.tile([128, O], f16, tag="m_q")
            nc.vector.tensor_max(m, gq[0], gq[1])
            nc.vector.tensor_max(m, m, gq[2])
            nc.vector.tensor_max(m, m, gq[3])
            t2 = wk_pool.tile([128, O], f16, tag="t2_q")
            sq = wk_pool.tile([128, O], f16, tag="sq_q")
            nc.scalar.activation(out=t2, in_=gq[0], func=AF.Square)
            nc.scalar.activation(out=sq, in_=gq[1], func=AF.Square)
            nc.vector.tensor_add(t2, t2, sq)
            nc.scalar.activation(out=sq, in_=gq[2], func=AF.Square)
            nc.vector.tensor_add(t2, t2, sq)
            nc.scalar.activation(out=sq, in_=gq[3], func=AF.Square)
            nc.vector.tensor_add(t2, t2, sq)
            ct = wk_pool.tile([128, O], f16, tag="ct_q")
            nc.vector.tensor_mul(ct, c_pm[st][:, qs], t)
            for qi, srct in ((6, t), (7, t2), (8, ct)):
                col = qi * 8 + st
                nc.tensor.matmul(out=stats_ps[:, col:col + 1], lhsT=srct,
                                 rhs=ones16, start=True, stop=True)
            ptr = psum([128, 128], dtp=f16)
            nc.tensor.transpose(out=ptr, in_=m, identity=ident)
            nc.scalar.activation(
                out=m_cm[:, 2 * PTS + st * 128:2 * PTS + (st + 1) * 128],
                in_=ptr, func=AF.Copy)

        # ---------------- stats AllReduce #2 (q) ----------------
        ccq = persist.tile([128, 5], f32)
        nc.vector.tensor_reduce(out=ccq[:, 0:1], in_=stats_ps[:, 48:56],
                                axis=mybir.AxisListType.X, op=ALU.add)
        nc.vector.tensor_reduce(out=ccq[:, 1:2], in_=stats_ps[:, 56:64],
                                axis=mybir.AxisListType.X, op=ALU.add)
        nc.vector.tensor_reduce(out=ccq[:, 2:3], in_=stats_ps[:, 64:72],
                                axis=mybir.AxisListType.X, op=ALU.add)
        nc.vector.tensor_add(ccq[:, 3:4], csum[:, 8:9], csum[:, 9:10])
        nc.vector.tensor_add(ccq[:, 4:5], csum[:, 10:11], csum[:, 11:12])
        nc.sync.dma_start(out=cc2_in, in_=ccq)
        nc.gpsimd.collective_compute("AllReduce", mybir.AluOpType.add,
                                     replica_groups=RG,
                                     ins=[cc2_in], outs=[cc2_out])
        stats2 = persist.tile([128, 5], f32)
        nc.sync.dma_start(out=stats2, in_=cc2_out)
        finalize(2, stats2[:, 0:1], stats2[:, 1:2], stats2[:, 2:3],
                 stats2[:, 3:4], stats2[:, 4:5],
                 scl[:, 4:5], scl[:, 5:6])

        # ---------------- q gate + output ----------------
        qpre = wk_pool.tile([128, PTS], f16, tag="qpre")
        nc.vector.tensor_add(qpre, m_cm[:, 2 * PTS:3 * PTS],
                             c_cm[:, 2 * PTS:3 * PTS])
        q_sb = persist.tile([O, PTS], f32)
        nc.scalar.activation(out=q_sb, in_=qpre, func=AF.Tanh,
                             scale=scl[:, 4:5], bias=scl[:, 5:6])
        # return only delta = z*(q - h); the host adds exact f32 h back,
        # so the wire carries 1 byte/elem (e3m4) instead of 2.
        dfin = persist.tile([O, PTS], f32)
        nc.vector.tensor_sub(dfin, q_sb, hmy32)
        nc.vector.tensor_mul(dfin, dfin, z_sb)
        out8 = persist.tile([O, PTS], f8o)
        nc.scalar.activation(out=out8, in_=dfin, func=AF.Copy)
        nc.sync.dma_start(out=outm, in_=out8)
        # gather every core's slab everywhere; the host then fetches the
        # full output from a single device (one D2H round trip).
        nc.gpsimd.collective_compute("AllGather", mybir.AluOpType.bypass,
                                     replica_groups=RG8,
                                     ins=[outm], outs=[out_st])
        nc.sync.dma_start(out=out_io, in_=out_st)

    nc.compile()
    return nc


def _make_runner():
    """Build the Bass program, AOT-compile the sharded executable ONCE,
    and return a fast per-call closure (numpy prep + dispatch + gather)."""
    import jax
    import jax.numpy as jnp
    from jax.sharding import Mesh, PartitionSpec
    from jax.experimental.shard_map import shard_map
    from concourse import mybir
    from concourse.bass2jax import (_bass_exec_p, install_neuronx_cc_hook,
                                    partition_id_tensor, fast_dispatch_compile)

    nc = _build_program()
    install_neuronx_cc_hook()

    partition_name = nc.partition_id_tensor.name if nc.partition_id_tensor else None
    in_names, out_names, out_avals = [], [], []
    for alloc in nc.m.functions[0].allocations:
        if not isinstance(alloc, mybir.MemoryLocationSet):
            continue
        name = alloc.memorylocations[0].name
        if alloc.kind == "ExternalInput":
            if name != partition_name and name != (
                    nc.dbg_addr.name if nc.dbg_addr is not None else None):
                in_names.append(name)
        elif alloc.kind == "ExternalOutput":
            out_names.append(name)
            shape = tuple(alloc.tensor_shape)
            out_avals.append(jax.core.ShapedArray(shape, mybir.dt.np(alloc.dtype)))
    n_params = len(in_names)
    n_outs = len(out_avals)
    # The kernel writes every element of its outputs (the final AllGather
    # fills out_io completely), so no donated zero output buffers are
    # passed — PJRT-allocated uninit results are fine.
    in_names_all = list(in_names)
    if nc.dbg_addr is not None:
        in_names_all.append(nc.dbg_addr.name)
    if partition_name is not None:
        in_names_all.append(partition_name)

    def _body(*args):
        operands = list(args)
        if nc.dbg_addr is not None:
            operands.append(jnp.zeros((1, 2), jnp.uint32))
        if partition_name is not None:
            operands.append(partition_id_tensor())
        outs = _bass_exec_p.bind(
            *operands, out_avals=tuple(out_avals),
            in_names=tuple(in_names_all), out_names=tuple(out_names),
            lowering_input_output_aliases=(), sim_require_finite=True,
            sim_require_nnan=True, nc=nc)
        return tuple(outs)

    devices = jax.devices()[:NCORES]
    assert len(devices) == NCORES
    mesh = Mesh(np.asarray(devices), ("core",))
    in_specs = (PartitionSpec("core"),) * n_params
    out_specs = (PartitionSpec("core"),) * len(out_names)
    jitted = jax.jit(
        shard_map(_body, mesh=mesh, in_specs=in_specs, out_specs=out_specs,
                  check_rep=False),
        keep_unused=True)

    # per-input global (concatenated over cores) shapes and dtypes
    in_shapes = {}
    for alloc in nc.m.functions[0].allocations:
        if not isinstance(alloc, mybir.MemoryLocationSet):
            continue
        name = alloc.memorylocations[0].name
        if name in in_names:
            in_shapes[name] = (tuple(alloc.tensor_shape),
                               mybir.dt.np(alloc.dtype))

    f16, f32 = np.float16, np.float32
    f8o_np = mybir.dt.np(mybir.dt.float8e3)          # ml_dtypes.float8_e3m4
    # decode LUT for the e3m4 output delta
    dec_lut = np.arange(256, dtype=np.uint8).view(f8o_np).astype(f32)

    def prep(h, x, pc, Wz, bz, Wr, br, Wq, bq):
        """Build the concatenated (8*rows, cols) global input arrays."""
        # --- weights: sharded 1/8 per core, so the global concat is just
        # one full copy of each weight matrix ---
        Wq_m = Wq.copy()
        Wq_m[:, 3:3 + H] = 0.0
        WT = np.concatenate([Wz.T, Wr.T, Wq_m.T], axis=1)        # [387, 384]
        wtsh = WT[3:387].astype(f16)                             # [384, 384]
        wtg1 = WT[0:3].astype(f16)                               # [3, 384]
        wqsh = np.ascontiguousarray(Wq[:, 3:3 + H].T).astype(f16)
        bcol1 = np.stack([bz, br, bq], axis=1).astype(f32)       # [128, 3]
        brow1 = np.concatenate([bz, br, bq])[None, :].astype(f32)

        # --- per-core point shards (single strided-copy fills) ---
        feat = np.empty((B, GROUP, 384, PTS), f16)
        feat[:, :, 0:H] = h.reshape(B, H, GROUP, PTS).transpose(0, 2, 1, 3)
        feat[:, :, H:H + D] = x.reshape(B, D, GROUP, PTS).transpose(0, 2, 1, 3)
        feat = feat.reshape(NCORES * 384, PTS)

        sq = (pc * pc).sum(axis=1, keepdims=True)                # [2,1,4096]
        pca_b = np.concatenate([pc, sq], axis=1)                 # [2,4,4096]
        pcam = np.ascontiguousarray(
            pca_b.reshape(B, 4, GROUP, PTS).transpose(0, 2, 1, 3), dtype=f32)
        pct = np.empty((B, GROUP, 4, PTS), f32)
        pct[:, :, 0:3] = -2.0 * pcam[:, :, 0:3]
        pct[:, :, 3] = 1.0

        vals = {
            "feat": feat,
            "pct": pct.reshape(NCORES * 4, PTS),
            "pcm": pcam.reshape(NCORES * 4, PTS),
            "wtsh": wtsh, "wqsh": wqsh,
            "wtg": np.tile(wtg1, (NCORES, 1)),
            "bcol": np.tile(bcol1, (NCORES, 1)),
            "brow": np.tile(brow1, (NCORES, 1)),
        }
        return [vals[name] for name in in_names]

    # AOT compile with representative (zero) inputs
    sample_in = []
    for name in in_names:
        shape, dtp = in_shapes[name]
        sample_in.append(np.zeros((NCORES * shape[0], *shape[1:]), dtp))
    compiled = fast_dispatch_compile(
        lambda: jitted.lower(*sample_in).compile())

    out_idx = out_names.index("out")

    def run(h, x, pc, Wz, bz, Wr, br, Wq, bq):
        concat_in = prep(h, x, pc, Wz, bz, Wr, br, Wq, bq)
        outs = compiled(*concat_in)
        # every device holds the full output after the final AllGather;
        # fetch only one shard (one D2H round trip).
        shard = outs[out_idx].addressable_shards[0].data
        o = np.asarray(shard)                                    # e3m4 delta
        delta = dec_lut[o.view(np.uint8).reshape(NCORES, O, PTS)]
        delta = delta.reshape(B, GROUP, O, PTS).transpose(0, 2, 1, 3)
        return delta.reshape(B, H, S) + h

    return run


def kernel(h, x, pc, Wz, bz, Wr, br, Wq, bq):
    if "run" not in _CACHE:
        _CACHE["run"] = _make_runner()
    return _CACHE["run"](h, x, pc, Wz, bz, Wr, br, Wq, bq)
